# revision 2
# baseline (speedup 1.0000x reference)
"""Trainium2 Bass kernel for nn_Net_16999480558201 (gnn_message_passing), v3.

Model (reference):
    feats = [x_graph | x_m[m_ids] | x_job[job_idx]]          # [N, 4H]
    h  = relu(feats @ W0 + b0); h = relu(h @ W1 + b1)
    s  = (h @ W2 + b2)[:, 0]                                  # [N]
    -> (argmax(s), softmax(s)[idx], log_softmax(s)[idx], entropy)

Strategy (8 NeuronCores, data-parallel over N):
  * Host shards the N candidates and builds each core's [N, 4H] feature rows
    (per the sharding hint): the varying 2H half ships as an fp8 plane
    [128, 49*2*512] (per 512-tile: 512 x_m cols then 512 x_job cols),
    features on partitions.  The uniform x_graph half collapses into
    c = x_graph @ W0[:2H] + b0.
  * Device, per 1024-candidate supertile: fp8 DoubleRow matmuls contract
    both W0 halves at once (K=256) into a 2-bank PSUM tile, one 1024-wide
    relu(+c) -> h0 fp16, W1 fp16 matmuls, relu(+b1) -> h1, and DoubleRow
    "score-pair" matmuls (lhsT = two h1 128-blocks, rhs = [w2|0 ; 0|w2])
    put scores for 256 candidates per matmul into a [128, 196] PSUM bank.
  * The two relu passes rotate across Activation / Pool(GPSIMD) / DVE.
  * Supertiles are software-pipelined: each emission round carries stage A
    for supertile r, stage B for r-1, scores for r-2, so the PE queue never
    waits on a relu.
  * Per-core softmax partials (max, sum(exp), sum(s*exp), argmax) reduce on
    device; the 8x4 scalars combine on the host.
"""
import sys

if "/opt/trn_rl_repo" not in sys.path:
    sys.path.insert(0, "/opt/trn_rl_repo")

import numpy as np
import ml_dtypes

H = 128
N = 200000
M = 1000
J = 5000
NCORES = 8
PER = N // NCORES            # 25000 rows per core
T = 512                      # candidates per matmul (PSUM bank)
TILES = 49
NPAD = TILES * T             # 25088
ST = 25                      # supertiles of 2 tiles (last has 1)
SCOLS = TILES * 4            # 196 score columns ([128, 196] layout)
NEG_BIG = -1.0e30
IOTA_BASE = 32768
F8 = ml_dtypes.float8_e4m3

# scheduling knobs (sweepable)
Z_BUFS = 3
Z0_BUFS = 3    # 512-wide z0 tiles when SPLIT_Z
Z1_BUFS = 4    # 512-wide z1 tiles when SPLIT_Z
SPLIT_Z = False
B_LAG = 3
C_LAG = 5
PREFETCH = 4
STAGE = "full"   # "mlp" = skip softmax tail (bisect aid)
H0_ROT = "AD"
H1_ROT = "SHARED"

_CACHE = {}


def _build():
    import concourse.bacc as bacc
    import concourse.bass_isa as bass_isa
    import concourse.mybir as mybir
    import concourse.tile as tile
    from contextlib import ExitStack

    FP8 = mybir.dt.float8e4
    FP16 = mybir.dt.float16
    F32 = mybir.dt.float32
    I32 = mybir.dt.int32
    AF = mybir.ActivationFunctionType
    ALU = mybir.AluOpType
    AX = mybir.AxisListType
    DR = mybir.MatmulPerfMode.DoubleRow

    nc = bacc.Bacc("TRN2", target_bir_lowering=False, debug=False)

    feats_d = nc.dram_tensor("feats", [128, 2 * TILES, T], FP8,
                             kind="ExternalInput")
    w0dr_d = nc.dram_tensor("w0dr", [128, 2 * H], FP8, kind="ExternalInput")
    w1_d = nc.dram_tensor("w1", [H, H], FP16, kind="ExternalInput")
    w2p_d = nc.dram_tensor("w2p", [H, 4], FP8, kind="ExternalInput")
    cvec_d = nc.dram_tensor("cvec", [H, 1], F32, kind="ExternalInput")
    b1_d = nc.dram_tensor("b1", [H, 1], F32, kind="ExternalInput")
    out_d = nc.dram_tensor("out", [1, 4], F32, kind="ExternalOutput")

    def _emit(tc, ctx):
        cpool = ctx.enter_context(tc.tile_pool(name="consts", bufs=1))
        fpool = ctx.enter_context(tc.tile_pool(name="feats", bufs=5))
        h0pool = ctx.enter_context(tc.tile_pool(name="h0", bufs=5))
        h1pool = ctx.enter_context(tc.tile_pool(name="h1", bufs=4))
        rpool = ctx.enter_context(tc.tile_pool(name="red", bufs=1))
        if SPLIT_Z:
            z0pool = ctx.enter_context(
                tc.tile_pool(name="z0", bufs=Z0_BUFS, space="PSUM"))
            z1pool = ctx.enter_context(
                tc.tile_pool(name="z1", bufs=Z1_BUFS, space="PSUM"))
        else:
            z0pool = z1pool = ctx.enter_context(
                tc.tile_pool(name="z", bufs=Z_BUFS, space="PSUM"))
        psc = ctx.enter_context(tc.tile_pool(name="psc", bufs=1, space="PSUM"))

        # ---- constant loads ----
        w0dr = cpool.tile([128, 2, H], FP8)
        nc.sync.dma_start(out=w0dr[:, :, :], in_=w0dr_d[:, :])
        w1_sb = cpool.tile([H, H], FP16)
        nc.sync.dma_start(out=w1_sb[:, :], in_=w1_d[:, :])
        w2p_sb = cpool.tile([H, 2, 2], FP8)
        nc.sync.dma_start(out=w2p_sb[:, :, :], in_=w2p_d[:, :])
        c_sb = cpool.tile([H, 1], F32)
        nc.sync.dma_start(out=c_sb[:, :], in_=cvec_d[:, :])
        b1_sb = cpool.tile([H, 1], F32)
        nc.sync.dma_start(out=b1_sb[:, :], in_=b1_d[:, :])

        # iota constant (IOTA_BASE - (col*128 + row)), off the critical tail
        iota32 = rpool.tile([128, SCOLS], I32)
        nc.gpsimd.iota(iota32[:, :], pattern=[[128, SCOLS]], base=0,
                       channel_multiplier=1)
        iotaf = rpool.tile([128, SCOLS], F32)
        nc.vector.tensor_scalar(iotaf[:, :], iota32[:, :], -1.0, float(IOTA_BASE),
                                op0=ALU.mult, op1=ALU.add)

        # ---- scores PSUM bank, pre-filled with -BIG for padding ----
        psc_t = psc.tile([128, SCOLS], F32)
        nc.vector.memset(psc_t[:, :], NEG_BIG)

        # warm the Exp activation table so the tail doesn't pay the load
        warm = rpool.tile([128, 1], F32)
        nc.vector.memset(warm[:, :], 0.0)
        warm2 = rpool.tile([128, 1], F32)
        nc.scalar.activation(warm2[:, :], warm[:, :], AF.Exp)

        # stage-aware relu engine rotation (Act 0.91 / DVE 1.16 / Pool 1.48
        # ns/col)
        shared = (H1_ROT == "SHARED")
        H0R = list(H0_ROT)
        H1R = H0R if shared else list(H1_ROT)
        ctr = {"h0": 0, "h1": 0}
        K0, K1 = "h0", ("h0" if shared else "h1")

        def relu_one(i, out, in_, bias, rot):
            r = rot[i % len(rot)]
            if r == "A":
                nc.scalar.activation(out, in_, AF.Relu, bias=bias)
            elif r == "P":
                nc.gpsimd.tensor_scalar(out, in_, bias, 0.0,
                                        op0=ALU.add, op1=ALU.max)
            else:
                nc.vector.tensor_scalar(out, in_, bias, 0.0,
                                        op0=ALU.add, op1=ALU.max)

        def relu_op(i, out, in_, bias, w, rot):
            relu_one(i, out[:, 0:w], in_[:, 0:w], bias, rot)

        # ---- software-pipelined supertile rounds ----
        # supertile r = 2 PSUM-bank tiles (1024 cands; the last has 512).
        # slab s = 2 supertiles per DMA.
        # stage A(r): z0 = W0dr x feats (DoubleRow); h0 = relu(z0+c)
        # stage B(r): z1 = W1 x h0; h1 = relu(z1+b1)
        # stage C(r): score-pair matmuls from h1 into psc
        fs = {}
        h0t = {}
        h1t = {}
        NSLAB = ST                     # one slab per supertile

        def width(r):
            return 2 * T if r < ST - 1 else T

        def stageDMA(s):
            g0 = 4 * s
            ng = min(4, 2 * TILES - g0)
            fs[s] = fpool.tile([128, 4, T], FP8, tag="fs", name="fs")
            nc.sync.dma_start(out=fs[s][:, 0:ng, :],
                              in_=feats_d[:, g0: g0 + ng, :])

        def stageA(r):
            w = width(r)
            sl = fs[r]
            h0t[r] = h0pool.tile([128, 2 * T], FP16, tag="h0", name="h0")
            if SPLIT_Z:
                for i in range(w // T):
                    z0 = z0pool.tile([128, T], F32, tag="z0", name="z0")
                    nc.tensor.matmul(z0[:, :], w0dr[:, :, :],
                                     sl[:, 2 * i: 2 * i + 2, :],
                                     start=True, stop=True, perf_mode=DR)
                    relu_one(ctr[K0], h0t[r][:, i * T: (i + 1) * T], z0[:, :],
                             c_sb[:, :], H0R)
                    ctr[K0] += 1
            else:
                z0 = z0pool.tile([128, 2 * T], F32, tag="z")
                for i in range(w // T):
                    nc.tensor.matmul(z0[:, i * T: (i + 1) * T], w0dr[:, :, :],
                                     sl[:, 2 * i: 2 * i + 2, :],
                                     start=True, stop=True, perf_mode=DR)
                relu_op(ctr[K0], h0t[r], z0, c_sb[:, :], w, H0R)
                ctr[K0] += 1
            fs.pop(r)

        def stageB(r):
            w = width(r)
            h1t[r] = h1pool.tile([128, 8, 128], FP8, tag="h1", name="h1")
            if SPLIT_Z:
                for i in range(w // T):
                    z1 = z1pool.tile([128, T], F32, tag="z1", name="z1")
                    nc.tensor.matmul(z1[:, :], w1_sb[:, :],
                                     h0t[r][:, i * T: (i + 1) * T],
                                     start=True, stop=True)
                    relu_one(ctr[K1], h1t[r][:, 4 * i: 4 * (i + 1), :],
                             z1[:, :], b1_sb[:, :], H1R)
                    ctr[K1] += 1
            else:
                z1 = z1pool.tile([128, 2 * T], F32, tag="z")
                for i in range(w // T):
                    nc.tensor.matmul(z1[:, i * T: (i + 1) * T], w1_sb[:, :],
                                     h0t[r][:, i * T: (i + 1) * T],
                                     start=True, stop=True)
                relu_one(ctr[K1], h1t[r][:, 0: w // 128, :], z1[:, 0:w],
                         b1_sb[:, :], H1R)
                ctr[K1] += 1
            h0t.pop(r)

        def stageC(r):
            w = width(r)
            h1 = h1t[r]
            for p in range(w // 256):  # score pairs: 256 cands each
                col = r * 8 + 2 * p
                row0 = col * 128
                nr0 = max(0, min(128, PER - row0))
                nr1 = max(0, min(128, PER - row0 - 128))
                if nr0 == 0:
                    break
                if nr1 == 128:
                    nc.tensor.matmul(
                        psc_t[0:128, col: col + 2],
                        h1[:, 2 * p: 2 * p + 2, :],
                        w2p_sb[:, :, :],
                        start=True, stop=True, perf_mode=DR,
                    )
                else:
                    # partial tail: garbage rows must stay NEG_BIG, so write
                    # each block with its exact row count.
                    nc.tensor.matmul(
                        psc_t[0:nr0, col: col + 1],
                        h1[:, 2 * p, 0:nr0],
                        w2p_sb[:, 0:1, 0:1],
                        start=True, stop=True,
                    )
                    if nr1 > 0:
                        nc.tensor.matmul(
                            psc_t[0:nr1, col + 1: col + 2],
                            h1[:, 2 * p + 1, 0:nr1],
                            w2p_sb[:, 0:1, 0:1],
                            start=True, stop=True,
                        )
            h1t.pop(r)

        stageDMA(0)
        stageDMA(1)
        stageDMA(2)
        for r in range(ST + 2):
            s = r // 2 + 3
            if r % 2 == 0 and s < NSLAB:
                stageDMA(s)
            if r < ST:
                stageA(r)
            if 1 <= r < ST + 1:
                stageB(r - 1)
            if r >= 2:
                stageC(r - 2)

        # ---- on-device softmax partials over scores [128, SCOLS] (PSUM) ----
        sc_sb = psc_t

        if STAGE == "mlp":
            out_sb = rpool.tile([1, 4], F32)
            nc.vector.tensor_copy(out_sb[:, :], sc_sb[0:1, 0:4])
            nc.sync.dma_start(out=out_d[:, :], in_=out_sb[:, :])
            return

        rmax = rpool.tile([128, 1], F32)
        nc.vector.tensor_reduce(rmax[:, :], sc_sb[:, :], axis=AX.X, op=ALU.max)
        mxb = rpool.tile([128, 1], F32)
        nc.gpsimd.partition_all_reduce(mxb[:, :], rmax[:, :], 128,
                                       bass_isa.ReduceOp.max)
        negmx = rpool.tile([128, 1], F32)
        nc.vector.tensor_scalar(negmx[:, :], mxb[:, :], -1.0, None, op0=ALU.mult)

        # branch 1 (Act -> DVE): exp, then sum(s * e^s) fused
        expd = rpool.tile([128, SCOLS], F32)
        zrow = rpool.tile([128, 1], F32)
        nc.scalar.activation(expd[:, :], sc_sb[:, :], AF.Exp,
                             bias=negmx[:, :], accum_out=zrow[:, :])
        sxe = rpool.tile([128, SCOLS], F32)
        nc.vector.tensor_tensor(sxe[:, :], expd[:, :], sc_sb[:, :], op=ALU.mult)
        srow = rpool.tile([128, 1], F32)
        nc.vector.tensor_reduce(srow[:, :], sxe[:, :], axis=AX.X, op=ALU.add)
        # branch 2 (Pool): argmax candidate = (s == max) * iota, fused
        eqm = rpool.tile([128, SCOLS], F32)
        nc.vector.tensor_scalar(eqm[:, :], sc_sb[:, :], mxb[:, :], None,
                                op0=ALU.is_equal)
        cand = rpool.tile([128, SCOLS], F32)
        nc.vector.tensor_tensor(cand[:, :], eqm[:, :], iotaf[:, :], op=ALU.mult)
        crow = rpool.tile([128, 1], F32)
        nc.vector.tensor_reduce(crow[:, :], cand[:, :], axis=AX.X, op=ALU.max)
        idxn = rpool.tile([128, 1], F32)
        nc.gpsimd.partition_all_reduce(idxn[:, :], crow[:, :], 128,
                                       bass_isa.ReduceOp.max)
        zsum = rpool.tile([128, 1], F32)
        nc.gpsimd.partition_all_reduce(zsum[:, :], zrow[:, :], 128,
                                       bass_isa.ReduceOp.add)
        ssum = rpool.tile([128, 1], F32)
        nc.gpsimd.partition_all_reduce(ssum[:, :], srow[:, :], 128,
                                       bass_isa.ReduceOp.add)

        out_sb = rpool.tile([1, 4], F32)
        nc.vector.tensor_copy(out_sb[:, 0:1], mxb[0:1, :])
        nc.vector.tensor_copy(out_sb[:, 1:2], zsum[0:1, :])
        nc.vector.tensor_copy(out_sb[:, 2:3], ssum[0:1, :])
        nc.vector.tensor_copy(out_sb[:, 3:4], idxn[0:1, :])
        nc.sync.dma_start(out=out_d[:, :], in_=out_sb[:, :])

    with tile.TileContext(nc) as tc, ExitStack() as ctx:
        _emit(tc, ctx)

    nc.compile()
    return nc


def _get_nc():
    if "nc" not in _CACHE:
        _CACHE["nc"] = _build()
    return _CACHE["nc"]


def _prep_in_maps(x_graph, x_m, x_job, m_ids, job_idx, W0, b0, W1, b1, W2):
    x_m = np.asarray(x_m, np.float32)
    x_job = np.asarray(x_job, np.float32)
    W0 = np.asarray(W0, np.float32)
    x_graph = np.asarray(x_graph, np.float32)
    c = (x_graph @ W0[0: 2 * H] + np.asarray(b0, np.float32)).reshape(H, 1)
    w0dr = np.empty((128, 2, H), F8)
    w0dr[:, 0, :] = W0[2 * H: 3 * H].astype(F8)
    w0dr[:, 1, :] = W0[3 * H: 4 * H].astype(F8)
    w2 = np.asarray(W2, np.float32).reshape(H).astype(np.float16)
    w2p = np.zeros((H, 2, 2), F8)
    w2p[:, 0, 0] = w2.astype(F8)
    w2p[:, 1, 1] = w2.astype(F8)
    shared = {
        "w0dr": w0dr.reshape(128, 2 * H),
        "w1": np.asarray(W1, np.float32).astype(np.float16),
        "w2p": w2p.reshape(H, 4),
        "cvec": c.astype(np.float32),
        "b1": np.asarray(b1, np.float32).reshape(H, 1),
    }
    xmT = np.ascontiguousarray(x_m.T.astype(F8))    # [128, M]
    xjT = np.ascontiguousarray(x_job.T.astype(F8))  # [128, J]
    m_ids = np.asarray(m_ids).astype(np.int64)
    job_idx = np.asarray(job_idx).astype(np.int64)
    pad = np.zeros(NPAD - PER, np.int64)
    in_maps = []
    for k in range(NCORES):
        mk = np.concatenate([m_ids[k * PER: (k + 1) * PER], pad])
        jk = np.concatenate([job_idx[k * PER: (k + 1) * PER], pad])
        F = np.empty((128, TILES, 2, T), F8)
        F[:, :, 0, :] = xmT[:, mk].reshape(128, TILES, T)
        F[:, :, 1, :] = xjT[:, jk].reshape(128, TILES, T)
        in_maps.append({**shared, "feats": F.reshape(128, 2 * TILES, T)})
    return in_maps


def kernel(x_graph, x_m, x_job, m_ids, job_idx, W0, b0, W1, b1, W2, b2,
           _trace=False):
    from concourse.bass_utils import run_bass_kernel_spmd

    nc = _get_nc()
    in_maps = _prep_in_maps(x_graph, x_m, x_job, m_ids, job_idx,
                            W0, b0, W1, b1, W2)

    res = run_bass_kernel_spmd(nc, in_maps, list(range(NCORES)), trace=_trace)
    outs = np.stack([res.results[k]["out"][0] for k in range(NCORES)])
    if _trace:
        _CACHE["last_result"] = res

    mx = outs[:, 0].astype(np.float64)
    Z = outs[:, 1].astype(np.float64)
    S = outs[:, 2].astype(np.float64)
    lidx = (IOTA_BASE - outs[:, 3]).astype(np.int64)

    gm = mx.max()
    kstar = int(np.argmax(mx))
    w = np.exp(mx - gm)
    Zg = float((Z * w).sum())
    Sg = float((S * w).sum())
    lse = gm + np.log(Zg)
    entropy = lse - Sg / Zg
    idx = kstar * PER + int(lidx[kstar])
    logp = float(gm - lse)
    prob = float(np.exp(logp))
    return (np.int32(idx), np.float32(prob), np.float32(logp),
            np.float32(entropy))
